# revision 70
# baseline (speedup 1.0000x reference)
"""Trainium2 Bass kernel for nn_DFlashDraftModel (dense draft transformer).

Sharding: tensor-parallel over heads across 8 cores (2 Q heads + 1 KV head
per core), MLP columns/rows 8-way, fc (target_hidden projection) row-sharded
with one AllGather, 2 AllReduces per layer for the (tiny) hidden stream.

On-device layout is feature-major ("transposed"): activations are stored as
[feature_partition, token] so every matmul consumes weights [in, out] directly
as the stationary lhsT operand and no activation transposes are needed except
V (PE-transposed per 128-row tile for the PV matmul).
"""

import numpy as np
import ml_dtypes

import concourse.bass as bass
import concourse.tile as tile
from concourse import bacc, mybir
from concourse.bass_utils import run_bass_kernel_spmd
from concourse.masks import make_identity
from contextlib import ExitStack

AF = mybir.ActivationFunctionType
ALU = mybir.AluOpType
F32 = mybir.dt.float32
BF16 = mybir.dt.bfloat16
BF = ml_dtypes.bfloat16

# model dims
B, Q, CTX, L, H = 2, 32, 2048, 4, 2048
NH, NKV, HD, INTER = 16, 8, 128, 6144
KV = CTX + Q           # 2080
KT = H // 128          # 16 feature tiles
FT = 8192 // 128       # 64 fc contraction tiles
IT = (INTER // 8) // 128  # 6 inter tiles per core
XC = B * Q             # 64 hidden-stream columns
COLS = B * KV          # 4160 kv columns
RWS = (B * CTX) // 8   # 512 fc rows per core
NCORES = 8
EPS = 1e-6
THETA = 1000000.0
SCALE = HD ** -0.5
RG = [list(range(NCORES))]

TRACE = False
FAKE_COLL = False  # replace collectives with local DMAs (TimelineSim analysis)
_CACHE = {}


def _bcol(b, j):
    """column offset/width in the [*, 4160] kv panel for batch b, n-tile j"""
    off = b * KV + j * 512
    w = 512 if j < 4 else KV - CTX  # tail tile = the 32 x-columns
    return off, w


def build_program():
    nc = bacc.Bacc("TRN2", target_bir_lowering=False, debug=False,
                   enable_asserts=True, num_devices=NCORES)

    # ---------------- I/O ----------------
    thT_h = nc.dram_tensor("thT", [8192, RWS], BF16, kind="ExternalInput")
    fcw_h = nc.dram_tensor("fcw", [16, 128, 8192], BF16, kind="ExternalInput")
    hT0_h = nc.dram_tensor("hT0", [H, XC], F32, kind="ExternalInput")
    wq_h = nc.dram_tensor("wq", [L, 2, 128, 2048], BF16, kind="ExternalInput")
    wk_h = nc.dram_tensor("wk", [L, 128, 2048], BF16, kind="ExternalInput")
    wv_h = nc.dram_tensor("wv", [L, 128, 2048], BF16, kind="ExternalInput")
    wo_h = nc.dram_tensor("wo", [L, 16, 128, 256], BF16, kind="ExternalInput")
    gw_h = nc.dram_tensor("gw", [L, 6, 128, 2048], BF16, kind="ExternalInput")
    uw_h = nc.dram_tensor("uw", [L, 6, 128, 2048], BF16, kind="ExternalInput")
    dw_h = nc.dram_tensor("dw", [L, 16, 128, 768], BF16, kind="ExternalInput")
    csk_h = nc.dram_tensor("csk", [128, COLS], BF16, kind="ExternalInput")
    csn_h = nc.dram_tensor("csn", [128, COLS], BF16, kind="ExternalInput")
    csq_h = nc.dram_tensor("csq", [128, XC], BF16, kind="ExternalInput")
    csqn_h = nc.dram_tensor("csqn", [128, XC], BF16, kind="ExternalInput")
    ln1_h = nc.dram_tensor("ln1w", [128, L * KT], F32, kind="ExternalInput")
    ln2_h = nc.dram_tensor("ln2w", [128, L * KT], F32, kind="ExternalInput")
    hnw_h = nc.dram_tensor("hnw", [128, KT], F32, kind="ExternalInput")
    fnw_h = nc.dram_tensor("fnw", [128, KT], F32, kind="ExternalInput")
    qnw_h = nc.dram_tensor("qnw", [128, L], F32, kind="ExternalInput")
    knw_h = nc.dram_tensor("knw", [128, L], F32, kind="ExternalInput")
    outT_h = nc.dram_tensor("outT", [H, XC], F32, kind="ExternalOutput")

    with tile.TileContext(nc) as tc, ExitStack() as ctx:
        # ---------------- pools ----------------
        pre = ctx.enter_context(tc.tile_pool(name="pre", bufs=1))
        dram = ctx.enter_context(tc.tile_pool(name="dram", bufs=1, space="DRAM"))
        arp = ctx.enter_context(tc.tile_pool(name="arp", bufs=2, space="DRAM"))
        stats = ctx.enter_context(tc.tile_pool(name="stats", bufs=1))
        temps = ctx.enter_context(tc.tile_pool(name="temps", bufs=2))
        # psum pools: 2 + 2 + 3 + 1 = 8 banks
        mmp = ctx.enter_context(tc.tile_pool(name="mmp", bufs=2, space="PSUM"))
        mm64 = ctx.enter_context(tc.tile_pool(name="mm64", bufs=3, space="PSUM"))
        scp = ctx.enter_context(tc.tile_pool(name="scp", bufs=2, space="PSUM"))
        ssqp = ctx.enter_context(tc.tile_pool(name="ssqp", bufs=1, space="PSUM"))

        # ---------------- constants / small persistent ----------------
        ones_bf = pre.tile([128, 1], BF16, name="ones_bf")
        nc.vector.memset(ones_bf, 1.0)
        zb = pre.tile([128, 1], F32, name="zb")
        nc.vector.memset(zb, 0.0)
        epsb = pre.tile([1, 1], F32, name="epsb")
        nc.vector.memset(epsb, EPS)
        ident = pre.tile([128, 128], BF16, name="ident")
        make_identity(nc, ident)
        csq = pre.tile([128, XC], BF16, name="csq")
        csqn = pre.tile([128, XC], BF16, name="csqn")
        ln1 = pre.tile([128, L * KT], F32, name="ln1")
        ln2 = pre.tile([128, L * KT], F32, name="ln2")
        hnw = pre.tile([128, KT], F32, name="hnw")
        nc.scalar.dma_start(out=hnw, in_=hnw_h.ap())
        fnw = pre.tile([128, KT], F32, name="fnw")
        qnw = pre.tile([128, L], F32, name="qnw")
        knw = pre.tile([128, L], F32, name="knw")
        hT = pre.tile([128, KT * XC], F32, name="hT")  # residual stream (col k*64+x)

        # th allgather split into two feature halves so the second collective
        # overlaps the first half's SBUF spread (and layer-0 K/V start).
        th_loc2 = [dram.tile([H // 2, RWS], BF16, name=f"th_loc{i}")
                   for i in range(2)]
        th_all2 = [dram.tile([NCORES * H // 2, RWS], BF16, name=f"th_all{i}",
                             addr_space="Shared") for i in range(2)]

        def coll(kind, op, in_t, out_t):
            if FAKE_COLL:
                nc.sync.dma_start(out=out_t[0:in_t.shape[0], :], in_=in_t)
            else:
                nc.gpsimd.collective_compute(
                    kind, op, replica_groups=RG,
                    ins=[in_t.opt()], outs=[out_t.opt()])

        # ----- helper: column RMS stats -> broadcast 1/rms tile [128, w] -----
        def rms_bcast(srcs, w, div, nm):
            """srcs: list of [128, w] APs whose squares sum over partitions"""
            ssq = ssqp.tile([1, 512], F32, name=f"ssq_{nm}", tag="ssq")
            n = len(srcs)
            for i, ap in enumerate(srcs):
                sq = temps.tile([128, w], BF16, name=f"sq_{nm}_{i}", tag="sq512",
                                bufs=1)
                nc.vector.tensor_mul(sq, ap, ap)
                nc.tensor.matmul(ssq[:, :w], ones_bf[:, 0:1], sq,
                                 start=(i == 0), stop=(i == n - 1))
            nc.scalar.activation(ssq[:, :w], ssq[:, :w], AF.Sqrt,
                                 bias=epsb[:, 0:1], scale=1.0 / div)
            rc = stats.tile([1, w], F32, name=f"rc_{nm}", tag="rs2")
            nc.vector.reciprocal(rc, ssq[:, :w])
            rb = temps.tile([128, w], F32, name=f"rb_{nm}", tag="rstdb", bufs=1)
            nc.gpsimd.partition_broadcast(rb, rc)
            return rb

        # ----- helper: rope. cs/sn are cos/sin duplicated across both halves.
        # Walrus requires equal base partitions for 2-input SBUF ops, so
        # rotate_half is materialized with single-input cross-partition ops.
        def rope(src, dst, cs, sn, nm):
            w = src.shape[1]
            srot = temps.tile([128, w], BF16, name=f"srot_{nm}", tag="srot",
                              bufs=1)
            # rotate-half copies run on the idle GPSIMD engine
            nc.gpsimd.tensor_scalar_mul(srot[0:64, :], src[64:128, :], -1.0)
            nc.gpsimd.tensor_copy(srot[64:128, :], src[0:64, :])
            rt = temps.tile([128, w], BF16, name=f"rt_{nm}", tag="rtmp",
                            bufs=1)
            nc.vector.tensor_mul(rt, srot, sn)
            nc.vector.tensor_mul(dst, src, cs)
            nc.vector.tensor_add(dst, dst, rt)

        # ---------------- phase 1: fc matmul + hidden_norm ----------------
        with tc.tile_pool(name="fcp", bufs=1) as fcp, \
             tc.tile_pool(name="fcwp", bufs=2) as fcwp:
            panel = fcp.tile([128, FT * RWS], BF16, name="panel")
            # m=0 weight panel first so compute can start immediately
            fw0 = fcwp.tile([128, 8192], BF16, name="fcw0", tag="fcw")
            for q4 in range(4):
                nc.scalar.dma_start(
                    out=fw0[:, q4 * 2048:(q4 + 1) * 2048],
                    in_=fcw_h[0, :, q4 * 2048:(q4 + 1) * 2048])
            # input panel chunked across both queues so matmuls start early
            for k in range(FT):
                eng = nc.sync if k % 2 == 0 else nc.scalar
                eng.dma_start(out=panel[:, k * RWS:(k + 1) * RWS],
                              in_=thT_h[k * 128:(k + 1) * 128, :])
            ssq = ssqp.tile([1, 512], F32, name="fcssq", tag="ssq")
            thpre = []
            for m in range(KT):
                if m == 0:
                    fw = fw0
                else:
                    fw = fcwp.tile([128, 8192], BF16, name=f"fcw{m}", tag="fcw")
                    for q4 in range(4):
                        nc.scalar.dma_start(
                            out=fw[:, q4 * 2048:(q4 + 1) * 2048],
                            in_=fcw_h[m, :, q4 * 2048:(q4 + 1) * 2048])
                ps = mmp.tile([128, RWS], F32, name=f"fcps{m}", tag="mmp")
                for k in range(FT):
                    nc.tensor.matmul(ps, fw[:, k * 128:(k + 1) * 128],
                                     panel[:, k * RWS:(k + 1) * RWS],
                                     start=(k == 0), stop=(k == FT - 1))
                tp = fcp.tile([128, RWS], BF16, name=f"thpre{m}")
                nc.vector.tensor_copy(tp, ps)
                sq = temps.tile([128, RWS], BF16, name=f"fcsq{m}", tag="sq512",
                                bufs=1)
                nc.vector.tensor_mul(sq, tp, tp)
                nc.tensor.matmul(ssq, ones_bf[:, 0:1], sq,
                                 start=(m == 0), stop=(m == KT - 1))
                thpre.append(tp)
            nc.scalar.activation(ssq, ssq, AF.Sqrt, bias=epsb[:, 0:1],
                                 scale=1.0 / H)
            rc = stats.tile([1, RWS], F32, name="fcrc", tag="rs2")
            nc.vector.reciprocal(rc, ssq)
            rb = temps.tile([128, RWS], F32, name="fcrb", tag="rstdb", bufs=1)
            nc.gpsimd.partition_broadcast(rb, rc)
            for m in range(KT):
                t1 = temps.tile([128, RWS], BF16, name=f"fct{m}", tag="k1", bufs=1)
                nc.vector.tensor_mul(t1, thpre[m], rb)
                nc.vector.tensor_scalar_mul(t1, t1, hnw[:, m:m + 1])
                half, mh = divmod(m, KT // 2)
                nc.sync.dma_start(
                    out=th_loc2[half][mh * 128:(mh + 1) * 128, :], in_=t1)
                if m == KT // 2 - 1:
                    coll("AllGather", ALU.bypass, th_loc2[0], th_all2[0])
        coll("AllGather", ALU.bypass, th_loc2[1], th_all2[1])

        # table loads land on the scalar queue behind the fc weight stream
        nc.scalar.dma_start(out=csq, in_=csq_h.ap())
        nc.scalar.dma_start(out=csqn, in_=csqn_h.ap())
        nc.scalar.dma_start(out=ln1, in_=ln1_h.ap())
        nc.scalar.dma_start(out=ln2, in_=ln2_h.ap())
        nc.scalar.dma_start(out=fnw, in_=fnw_h.ap())
        nc.scalar.dma_start(out=qnw, in_=qnw_h.ap())
        nc.scalar.dma_start(out=knw, in_=knw_h.ap())
        nc.scalar.dma_start(out=hT.rearrange("p (k n) -> p k n", k=KT),
                            in_=hT0_h.ap().rearrange("(k p) n -> p k n", p=128))

        # ---------------- phase 2: big persistent SBUF ----------------
        big = ctx.enter_context(tc.tile_pool(name="big", bufs=1))
        wqkv = ctx.enter_context(tc.tile_pool(name="wqkv", bufs=8))
        wwop = ctx.enter_context(tc.tile_pool(name="wwop", bufs=2))
        wdp = ctx.enter_context(tc.tile_pool(name="wdp", bufs=6))
        attp = ctx.enter_context(tc.tile_pool(name="attp", bufs=3))
        mid = ctx.enter_context(tc.tile_pool(name="mid", bufs=2))
        arup = ctx.enter_context(tc.tile_pool(name="arup", bufs=1))

        # one big panel [128, k*(B*CTX) + b*CTX + pos] so each rank's spread
        # is a single large strided DMA per feature-half
        thsb_all = big.tile([128, KT * B * CTX], BF16, name="thsb_all")
        thsb = [thsb_all[:, k * B * CTX:(k + 1) * B * CTX] for k in range(KT)]
        kc = big.tile([128, COLS], BF16, name="kc")
        vrm = [big.tile([128, 17 * 128], BF16, name=f"vrm{b}") for b in range(B)]

        thsb3 = thsb_all.rearrange("p (k c) -> p k c", k=KT)
        for r in range(NCORES):
            b, j = divmod(r, 4)
            for half in range(2):
                eng = nc.sync if half == 0 else nc.scalar
                out3 = thsb3[:, half * (KT // 2):(half + 1) * (KT // 2),
                             b * CTX + j * 512: b * CTX + (j + 1) * 512]
                eng.dma_start(
                    out=out3,
                    in_=th_all2[half][r * (H // 2):(r + 1) * (H // 2), :]
                    .rearrange("(kh p) n -> p kh n", p=128))

        # ----- per-layer building blocks -----
        def hnorm(lw_ap, out_bf, nm):
            """out = rms_norm(h) * lnw  -> [128, KT*XC]"""
            sqb = temps.tile([128, KT * XC], BF16, name=f"sqb_{nm}",
                             tag="sq512", bufs=1)
            nc.vector.tensor_mul(sqb, hT, hT)
            ssq = ssqp.tile([1, 512], F32, name=f"hssq_{nm}", tag="ssq")
            for k in range(KT):
                nc.tensor.matmul(ssq[:, :XC], ones_bf[:, 0:1],
                                 sqb[:, k * XC:(k + 1) * XC],
                                 start=(k == 0), stop=(k == KT - 1))
            nc.scalar.activation(ssq[:, :XC], ssq[:, :XC], AF.Sqrt,
                                 bias=epsb[:, 0:1], scale=1.0 / H)
            rc = stats.tile([1, XC], F32, name=f"hrc_{nm}", tag="rs2")
            nc.vector.reciprocal(rc, ssq[:, :XC])
            rb = temps.tile([128, XC], F32, name=f"hrb_{nm}", tag="rstdb",
                            bufs=1)
            nc.gpsimd.partition_broadcast(rb, rc)
            # two whole-row ops with free-dim-broadcast APs
            h3 = hT.rearrange("p (k n) -> p k n", k=KT)
            o3 = out_bf.rearrange("p (k n) -> p k n", k=KT)
            rb_b = bass.AP(tensor=rb.tensor, offset=rb.offset,
                           ap=[rb.ap[0], [0, KT], rb.ap[1]])
            ln_b = bass.AP(tensor=lw_ap.tensor, offset=lw_ap.offset,
                           ap=[lw_ap.ap[0], lw_ap.ap[1], [0, XC]])
            nc.vector.tensor_tensor(out=o3, in0=h3, in1=rb_b, op=ALU.mult)
            nc.vector.tensor_tensor(out=o3, in0=o3, in1=ln_b, op=ALU.mult)

        def kv_tile(l, b, j, wks, wvs, nm):
            off, w = _bcol(b, j)

            def rhs(k):
                # tail tile reads x directly from xT (the kv_in concat)
                if j < 4:
                    return thsb[k][:, b * CTX + j * 512: b * CTX + j * 512 + w]
                return xT[:, k * XC + b * Q: k * XC + b * Q + w]

            # K projection
            ps = mmp.tile([128, w], F32, name=f"kps_{nm}", tag="mmp")
            for k in range(KT):
                nc.tensor.matmul(ps, wks[:, k * 128:(k + 1) * 128], rhs(k),
                                 start=(k == 0), stop=(k == KT - 1))
            kraw = temps.tile([128, w], BF16, name=f"kraw_{nm}", tag="kraw", bufs=1)
            nc.vector.tensor_copy(kraw, ps)
            rb = rms_bcast([kraw], w, HD, f"kn_{nm}")
            k1 = temps.tile([128, w], BF16, name=f"k1_{nm}", tag="k1", bufs=1)
            nc.vector.tensor_mul(k1, kraw, rb)
            nc.vector.tensor_scalar_mul(k1, k1, knw[:, l:l + 1])
            # cos/sin slices streamed from HBM (frees SBUF for weight prefetch)
            cst = temps.tile([128, w], BF16, name=f"cs_{nm}", tag="cst", bufs=2)
            nc.sync.dma_start(out=cst, in_=csk_h[:, off:off + w])
            snt = temps.tile([128, w], BF16, name=f"sn_{nm}", tag="snt", bufs=2)
            nc.sync.dma_start(out=snt, in_=csn_h[:, off:off + w])
            rope(k1, kc[:, off:off + w], cst, snt, nm)
            # V projection
            ps2 = mmp.tile([128, w], F32, name=f"vps_{nm}", tag="mmp")
            for k in range(KT):
                nc.tensor.matmul(ps2, wvs[:, k * 128:(k + 1) * 128], rhs(k),
                                 start=(k == 0), stop=(k == KT - 1))
            vtmp = temps.tile([128, w], BF16, name=f"vtmp_{nm}", tag="vtmp",
                              bufs=1)
            nc.vector.tensor_copy(vtmp, ps2)
            nch = 4 if j < 4 else 1
            for t in range(nch):
                cw = 128 if j < 4 else w
                Tg = j * 4 + t if j < 4 else 16
                tp = scp.tile([128, 128], BF16, name=f"vtp_{nm}_{t}", tag="sc")
                nc.tensor.transpose(tp[0:cw, :], vtmp[:, t * 128:t * 128 + cw],
                                    ident)
                nc.vector.tensor_copy(vrm[b][0:cw, Tg * 128:(Tg + 1) * 128],
                                      tp[0:cw, :])

        def kv_full(l, nm):
            # own tag: these live across the layer boundary (tail tiles of
            # layer l run after layer l-1's MLP), sharing a tag with the MLP
            # panels deadlocks the slot rotation.
            wks = wqkv.tile([128, 2048], BF16, name=f"wks_{nm}", tag="wkv", bufs=2)
            nc.scalar.dma_start(out=wks, in_=wk_h[l])
            wvs = wqkv.tile([128, 2048], BF16, name=f"wvs_{nm}", tag="wkv", bufs=2)
            nc.scalar.dma_start(out=wvs, in_=wv_h[l])
            for b in range(B):
                for j in range(4):
                    kv_tile(l, b, j, wks, wvs, f"{nm}_{b}_{j}")
            return wks, wvs

        xT = mid.tile([128, KT * XC], BF16, name="xT_init", tag="xT", bufs=1)
        interT = mid.tile([128, IT * XC], BF16, name="inter_init", tag="inter",
                          bufs=1)
        aru = arup.tile([128, KT * XC], F32, name="aru")

        kvw_next = kv_full(0, "l0")  # layer-0 ctx K/V (runs as soon as th lands)

        for l in range(L):
            nm = f"L{l}"
            # x = rms_norm(h, ln1) ; copy x into the kv panel gap columns
            hnorm(ln1[:, l * KT:(l + 1) * KT], xT, f"x1_{nm}")
            # q projection + per-head rms + rope
            qro = []
            for hh in range(2):
                wqs = []
                for h2 in range(2):
                    wq2 = wqkv.tile([128, 1024], BF16,
                                    name=f"wqs_{nm}{hh}_{h2}", tag="wqkv")
                    nc.scalar.dma_start(out=wq2,
                                      in_=wq_h[l, hh, :, h2 * 1024:(h2 + 1) * 1024])
                    wqs.append(wq2)
                ps = mm64.tile([128, XC], F32, name=f"qps_{nm}{hh}", tag="mm64")
                for k in range(KT):
                    nc.tensor.matmul(ps, wqs[k // 8][:, (k % 8) * 128:
                                                     (k % 8 + 1) * 128],
                                     xT[:, k * XC:(k + 1) * XC],
                                     start=(k == 0), stop=(k == KT - 1))
                qraw = temps.tile([128, XC], BF16, name=f"qraw_{nm}{hh}",
                                  tag="kraw", bufs=1)
                nc.vector.tensor_copy(qraw, ps)
                rb = rms_bcast([qraw], XC, HD, f"qn_{nm}{hh}")
                q1 = temps.tile([128, XC], BF16, name=f"q1_{nm}{hh}", tag="k1", bufs=1)
                nc.vector.tensor_mul(q1, qraw, rb)
                nc.vector.tensor_scalar_mul(q1, q1, qnw[:, l:l + 1])
                qq = attp.tile([128, XC], BF16, name=f"qro_{nm}{hh}",
                               tag=f"qro{hh}", bufs=2)
                rope(q1, qq, csq, csqn, f"q_{nm}{hh}")
                qro.append(qq)
            # tail kv tiles (depend on x)
            wks, wvs = kvw_next
            for b in range(B):
                kv_tile(l, b, 4, wks, wvs, f"t_{nm}_{b}")
            # attention: both heads share the kv head -> batch them per kv tile
            o_h = [attp.tile([128, XC], BF16, name=f"oh_{nm}{hh}",
                             tag=f"oh{hh}", bufs=2) for hh in range(2)]
            for b in range(B):
                ssum = mm64.tile([1, XC], F32, name=f"ssum_{nm}{b}",
                                 tag="mm64")
                oT = [mm64.tile([128, Q], F32, name=f"oT_{nm}{b}{hh}",
                                tag="mm64") for hh in range(2)]
                nt = 17
                for T in range(nt):
                    cnt = 128 if T < 16 else KV - CTX
                    koff = b * KV + T * 128
                    sc = scp.tile([128, XC], F32, name=f"sc_{nm}{b}{T}",
                                  tag="sc")
                    for hh in range(2):
                        nc.tensor.matmul(sc[0:cnt, hh * Q:(hh + 1) * Q],
                                         kc[:, koff:koff + cnt],
                                         qro[hh][:, b * Q:(b + 1) * Q],
                                         start=True, stop=True)
                    ex = attp.tile([128, XC], BF16, name=f"ex_{nm}{b}{T}",
                                   tag="exps")
                    nc.scalar.activation(ex[0:cnt, :], sc[0:cnt, :], AF.Exp,
                                         bias=zb[0:cnt, 0:1], scale=SCALE)
                    nc.tensor.matmul(ssum, ones_bf[0:cnt, 0:1], ex[0:cnt, :],
                                     start=(T == 0), stop=(T == nt - 1))
                    for hh in range(2):
                        nc.tensor.matmul(oT[hh],
                                         vrm[b][0:cnt, T * 128:(T + 1) * 128],
                                         ex[0:cnt, hh * Q:(hh + 1) * Q],
                                         start=(T == 0), stop=(T == nt - 1))
                rc = stats.tile([1, XC], F32, name=f"orc_{nm}{b}", tag="rs2")
                nc.vector.reciprocal(rc, ssum)
                rb = temps.tile([128, XC], F32, name=f"orb_{nm}{b}",
                                tag="rstdb", bufs=1)
                nc.gpsimd.partition_broadcast(rb, rc)
                for hh in range(2):
                    nc.vector.tensor_mul(o_h[hh][:, b * Q:(b + 1) * Q], oT[hh],
                                         rb[:, hh * Q:(hh + 1) * Q])
            # wo projection -> partial h update -> AllReduce
            for m in range(KT):
                wos = wwop.tile([128, 256], BF16, name=f"wos_{nm}{m}", tag="wwo")
                nc.scalar.dma_start(out=wos, in_=wo_h[l, m])
                wop = mm64.tile([128, XC], F32, name=f"wop_{nm}{m}", tag="mm64")
                for kh in range(2):
                    nc.tensor.matmul(wop, wos[:, kh * 128:(kh + 1) * 128],
                                     o_h[kh], start=(kh == 0), stop=(kh == 1))
                nc.vector.tensor_copy(aru[:, m * XC:(m + 1) * XC], wop)
            ar_in = arp.tile([H, XC], F32, name=f"ari_{nm}a", tag="arin")
            ar_out = arp.tile([H, XC], F32, name=f"aro_{nm}a", tag="arout",
                              addr_space="Shared")
            nc.sync.dma_start(out=ar_in.rearrange("(k p) n -> p k n", p=128),
                              in_=aru.rearrange("p (k n) -> p k n", k=KT))
            coll("AllReduce", ALU.add, ar_in, ar_out)
            # prefetch all MLP weight panels (independent of the AllReduce)
            gup = []
            for m in range(IT):
                ws = []
                for h2 in range(2):
                    g2 = wqkv.tile([128, 1024], BF16, name=f"gws_{nm}{m}_{h2}",
                                   tag="wqkv")
                    nc.scalar.dma_start(out=g2,
                                      in_=gw_h[l, m, :, h2 * 1024:(h2 + 1) * 1024])
                    u2 = wqkv.tile([128, 1024], BF16, name=f"uws_{nm}{m}_{h2}",
                                   tag="wqkv")
                    nc.scalar.dma_start(out=u2,
                                      in_=uw_h[l, m, :, h2 * 1024:(h2 + 1) * 1024])
                    ws.append((g2, u2))
                gup.append(ws)
            dwn = []
            for m in range(KT):
                dws = wdp.tile([128, 768], BF16, name=f"dws_{nm}{m}", tag="wdn")
                nc.scalar.dma_start(out=dws, in_=dw_h[l, m])
                dwn.append(dws)
            # next layer ctx K/V fills the AllReduce gap
            if l + 1 < L:
                kvw_next = kv_full(l + 1, f"l{l + 1}")
            nc.sync.dma_start(out=aru.rearrange("p (k n) -> p k n", k=KT),
                              in_=ar_out.rearrange("(k p) n -> p k n", p=128))
            nc.vector.tensor_add(hT, hT, aru)
            # MLP (x2 reuses the xT tile: all xT readers completed pre-AR)
            hnorm(ln2[:, l * KT:(l + 1) * KT], xT, f"x2_{nm}")
            for m in range(IT):
                gps = mm64.tile([128, XC], F32, name=f"gps_{nm}{m}", tag="mm64")
                for k in range(KT):
                    nc.tensor.matmul(gps, gup[m][k // 8][0][:, (k % 8) * 128:
                                                           (k % 8 + 1) * 128],
                                     xT[:, k * XC:(k + 1) * XC],
                                     start=(k == 0), stop=(k == KT - 1))
                ups = mm64.tile([128, XC], F32, name=f"ups_{nm}{m}", tag="mm64")
                for k in range(KT):
                    nc.tensor.matmul(ups, gup[m][k // 8][1][:, (k % 8) * 128:
                                                           (k % 8 + 1) * 128],
                                     xT[:, k * XC:(k + 1) * XC],
                                     start=(k == 0), stop=(k == KT - 1))
                sil = temps.tile([128, XC], BF16, name=f"sil_{nm}{m}",
                                 tag="kraw", bufs=1)
                nc.scalar.activation(sil, gps, AF.Silu, bias=zb[:, 0:1])
                nc.vector.tensor_mul(interT[:, m * XC:(m + 1) * XC], sil, ups)
            for m in range(KT):
                dws = dwn[m]
                dps = mm64.tile([128, XC], F32, name=f"dps_{nm}{m}", tag="mm64")
                for k in range(IT):
                    nc.tensor.matmul(dps, dws[:, k * 128:(k + 1) * 128],
                                     interT[:, k * XC:(k + 1) * XC],
                                     start=(k == 0), stop=(k == IT - 1))
                nc.vector.tensor_copy(aru[:, m * XC:(m + 1) * XC], dps)
            ar_in2 = arp.tile([H, XC], F32, name=f"ari_{nm}b", tag="arin")
            ar_out2 = arp.tile([H, XC], F32, name=f"aro_{nm}b", tag="arout",
                               addr_space="Shared")
            nc.sync.dma_start(out=ar_in2.rearrange("(k p) n -> p k n", p=128),
                              in_=aru.rearrange("p (k n) -> p k n", k=KT))
            coll("AllReduce", ALU.add, ar_in2, ar_out2)
            nc.sync.dma_start(out=aru.rearrange("p (k n) -> p k n", k=KT),
                              in_=ar_out2.rearrange("(k p) n -> p k n", p=128))
            nc.vector.tensor_add(hT, hT, aru)

        # final norm -> outT
        rb = rms_bcast([hT[:, k * XC:(k + 1) * XC] for k in range(KT)],
                       XC, H, "fin")
        fin = arup.tile([128, KT * XC], F32, name="fin", tag="aru")
        for k in range(KT):
            sl = fin[:, k * XC:(k + 1) * XC]
            nc.vector.tensor_mul(sl, hT[:, k * XC:(k + 1) * XC], rb)
            nc.vector.tensor_scalar_mul(sl, sl, fnw[:, k:k + 1])
        nc.sync.dma_start(out=outT_h.ap().rearrange("(k p) n -> p k n", p=128),
                          in_=fin.rearrange("p (k n) -> p k n", k=KT))

    nc.compile()
    return nc


def _prep_inputs(inputs):
    ne = np.asarray(inputs["noise_embedding"], np.float32)
    th = np.asarray(inputs["target_hidden"], np.float32)
    pos = np.asarray(inputs["position_ids"])
    fc = np.asarray(inputs["fc_w"], np.float32)
    wq = np.asarray(inputs["wq"], np.float32)
    wk = np.asarray(inputs["wk"], np.float32)
    wv = np.asarray(inputs["wv"], np.float32)
    wo = np.asarray(inputs["wo"], np.float32)
    gw = np.asarray(inputs["gate_w"], np.float32)
    uw = np.asarray(inputs["up_w"], np.float32)
    dw = np.asarray(inputs["down_w"], np.float32)

    fcw_t = np.ascontiguousarray(
        fc.reshape(64, 128, 16, 128).transpose(2, 1, 0, 3)
    ).reshape(16, 128, 8192).astype(BF)
    hT0 = np.ascontiguousarray(ne.reshape(XC, H).T).astype(np.float32)

    inv = 1.0 / (THETA ** (np.arange(0, HD, 2, dtype=np.float32) / HD))
    ang = pos.astype(np.float32)[:, :, None] * inv[None, None, :]  # [B,KV,64]
    # cos/sin duplicated across both 64-partition halves
    csk = np.empty((128, COLS), np.float32)
    csn = np.empty((128, COLS), np.float32)
    csq = np.empty((128, XC), np.float32)
    csqn = np.empty((128, XC), np.float32)
    for b in range(B):
        ck, sk = np.cos(ang[b]).T, np.sin(ang[b]).T
        csk[0:64, b * KV:(b + 1) * KV] = ck
        csk[64:128, b * KV:(b + 1) * KV] = ck
        csn[0:64, b * KV:(b + 1) * KV] = sk
        csn[64:128, b * KV:(b + 1) * KV] = sk
        cq, sq = np.cos(ang[b, KV - Q:]).T, np.sin(ang[b, KV - Q:]).T
        csq[0:64, b * Q:(b + 1) * Q] = cq
        csq[64:128, b * Q:(b + 1) * Q] = cq
        csqn[0:64, b * Q:(b + 1) * Q] = sq
        csqn[64:128, b * Q:(b + 1) * Q] = sq

    ln1w = np.ascontiguousarray(
        np.asarray(inputs["ln1_w"], np.float32).reshape(L, KT, 128)
        .transpose(2, 0, 1)).reshape(128, L * KT)
    ln2w = np.ascontiguousarray(
        np.asarray(inputs["ln2_w"], np.float32).reshape(L, KT, 128)
        .transpose(2, 0, 1)).reshape(128, L * KT)
    hnw = np.ascontiguousarray(
        np.asarray(inputs["hidden_norm_w"], np.float32).reshape(KT, 128).T)
    fnw = np.ascontiguousarray(
        np.asarray(inputs["final_norm_w"], np.float32).reshape(KT, 128).T)
    qnw = np.ascontiguousarray(np.asarray(inputs["qn_w"], np.float32).T)
    knw = np.ascontiguousarray(np.asarray(inputs["kn_w"], np.float32).T)

    flat = th.reshape(B * CTX, 8192)
    in_maps = []
    for c in range(NCORES):
        thT_c = np.ascontiguousarray(
            flat[c * RWS:(c + 1) * RWS].T).astype(BF)
        wq_c = np.ascontiguousarray(
            wq[:, :, c * 256:(c + 1) * 256]
            .reshape(L, 16, 128, 2, 128).transpose(0, 3, 2, 1, 4)
        ).reshape(L, 2, 128, 2048).astype(BF)
        wk_c = np.ascontiguousarray(
            wk[:, :, c * 128:(c + 1) * 128]
            .reshape(L, 16, 128, 128).transpose(0, 2, 1, 3)
        ).reshape(L, 128, 2048).astype(BF)
        wv_c = np.ascontiguousarray(
            wv[:, :, c * 128:(c + 1) * 128]
            .reshape(L, 16, 128, 128).transpose(0, 2, 1, 3)
        ).reshape(L, 128, 2048).astype(BF)
        wo_c = np.ascontiguousarray(
            wo[:, c * 256:(c + 1) * 256, :]
            .reshape(L, 2, 128, 16, 128).transpose(0, 3, 2, 1, 4)
        ).reshape(L, 16, 128, 256).astype(BF)
        gw_c = np.ascontiguousarray(
            gw[:, :, c * 768:(c + 1) * 768]
            .reshape(L, 16, 128, 6, 128).transpose(0, 3, 2, 1, 4)
        ).reshape(L, 6, 128, 2048).astype(BF)
        uw_c = np.ascontiguousarray(
            uw[:, :, c * 768:(c + 1) * 768]
            .reshape(L, 16, 128, 6, 128).transpose(0, 3, 2, 1, 4)
        ).reshape(L, 6, 128, 2048).astype(BF)
        dw_c = np.ascontiguousarray(
            dw[:, c * 768:(c + 1) * 768, :]
            .reshape(L, 6, 128, 16, 128).transpose(0, 3, 2, 1, 4)
        ).reshape(L, 16, 128, 768).astype(BF)
        in_maps.append(dict(
            thT=thT_c, fcw=fcw_t, hT0=hT0,
            wq=wq_c, wk=wk_c, wv=wv_c, wo=wo_c,
            gw=gw_c, uw=uw_c, dw=dw_c,
            csk=csk.astype(BF), csn=csn.astype(BF),
            csq=csq.astype(BF), csqn=csqn.astype(BF),
            ln1w=ln1w, ln2w=ln2w, hnw=hnw, fnw=fnw, qnw=qnw, knw=knw,
        ))
    return in_maps


_last_results = None


def kernel(**inputs):
    global _last_results
    if "nc" not in _CACHE:
        _CACHE["nc"] = build_program()
    nc = _CACHE["nc"]
    in_maps = _prep_inputs(inputs)
    res = run_bass_kernel_spmd(nc, in_maps, core_ids=list(range(NCORES)),
                               trace=TRACE)
    _last_results = res
    outT = res.results[0]["outT"]
    return np.ascontiguousarray(outT.T).reshape(B, Q, H).astype(np.float32)
